# revision 19
# baseline (speedup 1.0000x reference)
"""Trainium2 Bass kernel for nn_CausalAttention (N=4096, 8 heads, DH=32).

Strategy: head-parallel across 8 NeuronCores (1 head per core), tuned to be
ACT-engine bound (exp is only available on the scalar/ACT engine and its
~9.4M elements/core at 1 elem/lane/cycle @1.2GHz set a ~62us floor).

Per core:
  - PE warmup spin during the initial DMA window so the HAM clock gate
    reaches K=8/8 (2.4 GHz) before real matmuls start.
  - QK projections from channels-major inputs [256, 4096]; outputs written
    as 3x-replicated fp16 [96, N] (kT3/qT3) via bulk PSUM->SBUF
    tensor_scalar adds that fold the bias (one DVE op per 512-slice).
  - V projection in natural [key, d] orientation (kin slices as lhsT).
  - Scores S^T[k, q] = K Q^T in fp16, 3-way row-packed (tile_position via
    base partitions 0/32/64), 3 k-tiles per PSUM group (3 banks), strict
    causal: diagonal sub-tiles skip their fully-masked column prefix.
  - Max-free softmax: one exp activation per group [128, 512*nsub] on the
    ACT engine; nothing else runs on the scalar queue. Strict-causal 0/1
    mask applied post-exp on GpSimd. Garbage columns (masked prefixes) are
    never read: PV streams rhs starting at the prefix offset.
  - PV 2-way column-packed (col groups at partitions 0 and 64, 33-wide
    V|ones weights); denominator via the ones column; groups merged in the
    tail with one DVE add.
  - Normalization without transposes: per-block column sums -> reshape DMA
    [8,64] -> reciprocal -> [1,512] -> K=1 replicate matmul -> tensor_mul.
    Output stays in O^T layout [32, 4096] per core.
"""

import math

import numpy as np

import concourse.bass as bass
import concourse.mybir as mybir
from concourse import bacc
from concourse.tile import TileContext
from concourse.bass_utils import run_bass_kernel_spmd

# Problem constants (hardcoded per harness contract).
B, CQ, CK, CH, NH, H, W = 1, 256, 256, 256, 8, 64, 64
DH = CH // NH            # 32
N = H * W                # 4096
QB = 512                 # queries per block
NQB = N // QB            # 8
KT = 128                 # keys per k-tile
NKT = N // KT            # 32
GS = 3                   # k-tiles per score group (3 PSUM banks, 3-way row pack)
SCALE = 1.0 / math.sqrt(DH)
NWARM = 10               # PE warmup matmuls (HAM clock-gate warmup)

F32 = mybir.dt.float32
F32R = mybir.dt.float32r
F16 = mybir.dt.float16

_CACHED_NC = None


def _build():
    nc = bacc.Bacc("TRN2", target_bir_lowering=False, debug=False, num_devices=1)

    qin_d = nc.dram_tensor("qin", [CQ, N], F32, kind="ExternalInput")
    kin_d = nc.dram_tensor("kin", [CK, N], F32, kind="ExternalInput")
    wq_d = nc.dram_tensor("wqt3", [CQ, 96], F32, kind="ExternalInput")
    wk_d = nc.dram_tensor("wkt3", [CK, 96], F32, kind="ExternalInput")
    wv_d = nc.dram_tensor("wvt", [CK, DH], F32, kind="ExternalInput")
    bq_d = nc.dram_tensor("bq3", [96, 1], F32, kind="ExternalInput")
    bk_d = nc.dram_tensor("bk3", [96, 1], F32, kind="ExternalInput")
    bv_d = nc.dram_tensor("bvr", [128, 4 * DH], F32, kind="ExternalInput")
    out_d = nc.dram_tensor("out", [DH, N], F32, kind="ExternalOutput")

    # Strict-causal within-tile mask: tm[kk, qq] = 1.0 iff kk < qq.
    tm_np = (np.arange(128)[:, None] < np.arange(128)[None, :]).astype(np.float16)
    tm_d = nc.inline_tensor(tm_np, name="tmask")
    ones_d = nc.inline_tensor(np.ones((1, DH), dtype=np.float32), name="onesd")

    kin_ap = kin_d.ap().rearrange("(c p) n -> p c n", p=128).bitcast(F32R)
    qin_ap = qin_d.ap().rearrange("(c p) n -> p c n", p=128).bitcast(F32R)
    wk_ap = wk_d.ap().rearrange("(c p) m -> p c m", p=128).bitcast(F32R)
    wq_ap = wq_d.ap().rearrange("(c p) m -> p c m", p=128).bitcast(F32R)
    wv_ap = wv_d.ap().rearrange("(c p) m -> p c m", p=128).bitcast(F32R)

    with TileContext(nc) as tc:
        with (
            tc.tile_pool(name="constp", bufs=1) as constp,
            tc.tile_pool(name="bigp", bufs=1) as bigp,
            tc.tile_pool(name="workp", bufs=4) as workp,
            tc.tile_pool(name="spool", bufs=2, space="PSUM") as spool,
            tc.tile_pool(name="mpool", bufs=1, space="PSUM") as mpool,
        ):
            # ---- big SBUF tiles ----
            kin_sb = bigp.tile([128, 2, N], F32R, name="kin_sb")
            qin_sb = bigp.tile([128, 2, N], F32R, name="qin_sb")
            kT3 = bigp.tile([96, N], F16, name="kT3")    # 3x replicated k^T
            qT3 = bigp.tile([96, N], F16, name="qT3")    # 3x replicated q^T
            # v_all[kk, t, 0:DH] = v[128t+kk, :]; col DH is the ones column
            v_all = bigp.tile([128, NKT, 48], F16, name="v_all")
            warm = bigp.tile([32, 640], F16, name="warm")

            # ---- DMAs: weights + first slices first; kin on sync, qin on
            # gpsimd (cheap issue), nothing on the scalar queue ----
            wk_sb = constp.tile([128, 2, 96], F32R, name="wk_sb")
            nc.sync.dma_start(wk_sb[:], wk_ap)
            wq_sb = constp.tile([128, 2, 96], F32R, name="wq_sb")
            nc.sync.dma_start(wq_sb[:], wq_ap)
            slA = slice(0, QB)
            nc.sync.dma_start(kin_sb[:, :, slA], kin_ap[:, :, slA])
            nc.sync.dma_start(qin_sb[:, :, slA], qin_ap[:, :, slA])
            wv_sb = constp.tile([128, 2, DH], F32R, name="wv_sb")
            nc.gpsimd.dma_start(wv_sb[:], wv_ap)
            bq_sb = constp.tile([96, 1], F32, name="bq_sb")
            nc.gpsimd.dma_start(bq_sb[:], bq_d.ap())
            bk_sb = constp.tile([96, 1], F32, name="bk_sb")
            nc.gpsimd.dma_start(bk_sb[:], bk_d.ap())
            bv_sb = constp.tile([128, 4, DH], F32, name="bv_sb")
            nc.gpsimd.dma_start(bv_sb[:], bv_d.ap().rearrange("p (t d) -> p t d", t=4))
            tm_sb = constp.tile([128, 128], F16, name="tm_sb")
            nc.gpsimd.dma_start(tm_sb[:], tm_d.ap())
            ones_sb = constp.tile([1, DH], F32R, name="ones_sb")
            nc.gpsimd.dma_start(ones_sb[:], ones_d.ap().bitcast(F32R))
            nc.vector.memset(warm[:], 0.0)
            nc.vector.memset(v_all[:, :, DH : DH + 1], 1.0)
            for s in range(1, NQB):
                sl = slice(QB * s, QB * (s + 1))
                nc.gpsimd.dma_start(kin_sb[:, :, sl], kin_ap[:, :, sl])
                nc.gpsimd.dma_start(qin_sb[:, :, sl], qin_ap[:, :, sl])

            # ---- PE warmup: keep the array busy through the DMA window so
            # the HAM un-throttles before real matmuls arrive ----
            wm_ps = mpool.tile([128, 512], F32, name="wm_ps", tag="p")
            for i in range(NWARM):
                nc.tensor.matmul(
                    wm_ps[:], warm[0:32, 0:128], warm[0:32, 128:640],
                    start=(i == 0), stop=(i == NWARM - 1),
                )

            stage_q = []  # deferred tail stages, advanced one per score group

            def emit_kq(s):
                ksl = slice(QB * s, QB * (s + 1))
                pjk = mpool.tile([96, 512], F32, name="pjk", tag="p")
                for ch in range(2):
                    nc.tensor.matmul(
                        pjk[:], wk_sb[:, ch, :], kin_sb[:, ch, ksl],
                        start=(ch == 0), stop=(ch == 1),
                    )
                nc.vector.tensor_scalar_add(kT3[:, ksl], pjk[:], bk_sb[:])
                pjq = mpool.tile([96, 512], F32, name="pjq", tag="p")
                for ch in range(2):
                    nc.tensor.matmul(
                        pjq[:], wq_sb[:, ch, :], qin_sb[:, ch, ksl],
                        start=(ch == 0), stop=(ch == 1),
                    )
                nc.vector.tensor_scalar_add(qT3[:, ksl], pjq[:], bq_sb[:])

            def emit_v4(s):
                # 4 v-tiles of one slice into one PSUM tile: no per-tile DVE
                # round-trips on the single-bank "p" ring, one batched add
                pv4 = mpool.tile([128, 4, DH], F32, name="pv4", tag="p")
                for ti in range(4):
                    t = 4 * s + ti
                    nsl = slice(128 * t, 128 * (t + 1))
                    for ch in range(2):
                        nc.tensor.matmul(
                            pv4[:, ti, :], kin_sb[:, ch, nsl], wv_sb[:, ch, :],
                            start=(ch == 0), stop=(ch == 1),
                        )
                nc.vector.tensor_add(
                    v_all[:, 4 * s : 4 * s + 4, 0:DH], pv4[:], bv_sb[:]
                )

            def tail_b(st):
                cs8r = workp.tile([8, 64], F32, name="cs8r")
                nc.vector.reciprocal(cs8r[:], st["cs8"][:])
                csr = workp.tile([1, 512], F32R, name="csr")
                nc.sync.dma_start(csr[:], cs8r[:].bitcast(F32R))
                st.update(csr=csr)

            def tail_c(st):
                qb = st["qb"]
                rep_ps = mpool.tile([DH, 512], F32, name="rep_ps", tag="p")
                nc.tensor.matmul(
                    rep_ps[:], ones_sb[:], st["csr"][:], start=True, stop=True
                )
                out_sb = workp.tile([DH, 512], F32, name="out_sb")
                nc.vector.tensor_mul(out_sb[:], st["o_sb"][:], rep_ps[:])
                nc.sync.dma_start(
                    out_d.ap()[:, QB * qb : QB * (qb + 1)], out_sb[:]
                )

            def emit_qb(qb):
                nkt = 4 * (qb + 1)
                ngr = (nkt + GS - 1) // GS
                o_ps = mpool.tile([DH + 1, 512], F32, name="o_ps", tag="o")
                pends = []

                def flush_pv(pend):
                    tiles, p_sb = pend
                    for (u, j) in tiles:
                        o = max(0, 128 * j - QB * qb)
                        nc.tensor.matmul(
                            o_ps[:, o:512],
                            v_all[:, j, 0 : DH + 1],
                            p_sb[:, 512 * u + o : 512 * (u + 1)],
                            start=(j == 0),
                            stop=(j == nkt - 1),
                            skip_group_check=True,
                        )

                for g in range(ngr):
                    tiles = [(u, GS * g + u) for u in range(min(GS, nkt - GS * g))]
                    s_ps = spool.tile([128, GS * 512], F32, name="s_ps", tag="s")
                    for (u, j) in tiles:
                        o = max(0, 128 * j - QB * qb)
                        nc.tensor.matmul(
                            s_ps[:, 512 * u + o : 512 * (u + 1)],
                            kT3[32 * u : 32 * u + 32, 128 * j : 128 * (j + 1)],
                            qT3[32 * u : 32 * u + 32, QB * qb + o : QB * (qb + 1)],
                            start=True, stop=True,
                        )
                    p_sb = workp.tile([128, GS * 512], F16, name="p_sb", bufs=8)
                    nc.scalar.activation(
                        p_sb[:, 0 : 512 * len(tiles)],
                        s_ps[:, 0 : 512 * len(tiles)],
                        mybir.ActivationFunctionType.Exp,
                        scale=SCALE,
                    )
                    for (u, j) in tiles:
                        o = 128 * j - QB * qb
                        if o >= 0:  # strict-causal mask on the diagonal window
                            nc.gpsimd.tensor_mul(
                                p_sb[:, 512 * u + o : 512 * u + o + 128],
                                p_sb[:, 512 * u + o : 512 * u + o + 128],
                                tm_sb[:],
                            )
                    pends.append((tiles, p_sb))
                    if g == 0:
                        if qb == 0:
                            emit_kq(1)
                        emit_v4(qb)
                    elif g == 1 and qb + 2 < NQB:
                        emit_kq(qb + 2)
                    if len(pends) > 2:
                        flush_pv(pends.pop(0))
                    if stage_q:
                        stage_q.pop(0)()
                while pends:
                    flush_pv(pends.pop(0))

                # tail_a inline
                o_sb = workp.tile([DH, 512], F32, name="o_sb")
                cs_sb = workp.tile([1, 512], F32, name="cs_sb")
                nc.vector.tensor_copy(o_sb[:], o_ps[0:DH, :])
                # +1e-30 keeps q=0 (fully masked row) at 0 instead of NaN
                nc.vector.tensor_scalar_add(cs_sb[:], o_ps[DH : DH + 1, :], 1e-30)
                cs8 = workp.tile([8, 64], F32, name="cs8")
                nc.sync.dma_start(cs8[:], cs_sb[:])
                st = {"qb": qb, "o_sb": o_sb, "cs8": cs8}
                stage_q.append(lambda st=st: tail_b(st))
                stage_q.append(lambda: None)
                stage_q.append(lambda st=st: tail_c(st))

            emit_kq(0)
            for qb in range(NQB):
                emit_qb(qb)
            while stage_q:
                stage_q.pop(0)()

    nc.finalize()
    return nc


def _get_nc():
    global _CACHED_NC
    if _CACHED_NC is None:
        _CACHED_NC = _build()
    return _CACHED_NC


def _prep_in_maps(inputs):
    f = lambda a: np.ascontiguousarray(np.asarray(a, dtype=np.float32))
    query = f(inputs["query"]).reshape(CQ, N)
    key_feat = f(inputs["key_feat"]).reshape(CK, N)

    def wnorm(v, g):
        v = f(v)
        g = f(g)
        return g[:, None] * v / np.linalg.norm(v, axis=1, keepdims=True)

    wq = wnorm(inputs["vq"], inputs["gq"])
    wk = wnorm(inputs["vk"], inputs["gk"])
    wv = wnorm(inputs["vv"], inputs["gv"])
    bq, bk, bv = f(inputs["bq"]), f(inputs["bk"]), f(inputs["bv"])

    in_maps = []
    for c in range(NH):
        rows = slice(DH * c, DH * (c + 1))
        in_maps.append(
            {
                "qin": query,
                "kin": key_feat,
                "wqt3": np.ascontiguousarray(np.tile(wq[rows].T, (1, 3))),
                "wkt3": np.ascontiguousarray(np.tile(wk[rows].T, (1, 3))),
                "wvt": np.ascontiguousarray(wv[rows].T),
                "bq3": np.ascontiguousarray(np.tile(bq[rows], 3)[:, None]),
                "bk3": np.ascontiguousarray(np.tile(bk[rows], 3)[:, None]),
                "bvr": np.ascontiguousarray(np.tile(bv[rows][None, :], (128, 4))),
            }
        )
    return in_maps


def _run(inputs, trace=False, **kwargs):
    nc = _get_nc()
    in_maps = _prep_in_maps(inputs)
    res = None
    for attempt in range(3):
        try:
            res = run_bass_kernel_spmd(
                nc, in_maps, core_ids=list(range(NH)), trace=trace, **kwargs
            )
            break
        except Exception:
            if attempt == 2:
                raise

    out = np.empty((B, CH, H, W), dtype=np.float32)
    for c in range(NH):
        oc = res.results[c]["out"]  # [DH, N] (O^T layout)
        out[0, DH * c : DH * (c + 1)] = oc.reshape(DH, H, W)
    return out, res


def kernel(**inputs) -> np.ndarray:
    out, _ = _run(inputs, trace=False)
    return out


# revision 20
# speedup vs baseline: 1.1231x; 1.1231x over previous
"""Trainium2 Bass kernel for nn_CausalAttention (N=4096, 8 heads, DH=32).

Strategy: head-parallel across 8 NeuronCores (1 head per core), tuned to be
ACT-engine bound (exp is only available on the scalar/ACT engine and its
~9.4M elements/core at 1 elem/lane/cycle @1.2GHz set a ~62us floor).

Per core:
  - PE warmup spin during the initial DMA window so the HAM clock gate
    reaches K=8/8 (2.4 GHz) before real matmuls start.
  - QK projections from channels-major inputs [256, 4096]; outputs written
    as 3x-replicated fp16 [96, N] (kT3/qT3) via bulk PSUM->SBUF
    tensor_scalar adds that fold the bias (one DVE op per 512-slice).
  - V projection in natural [key, d] orientation (kin slices as lhsT).
  - Scores S^T[k, q] = K Q^T in fp16, 3-way row-packed (tile_position via
    base partitions 0/32/64), 3 k-tiles per PSUM group (3 banks), strict
    causal: diagonal sub-tiles skip their fully-masked column prefix.
  - Max-free softmax: one exp activation per group [128, 512*nsub] on the
    ACT engine; nothing else runs on the scalar queue. Strict-causal 0/1
    mask applied post-exp on GpSimd. Garbage columns (masked prefixes) are
    never read: PV streams rhs starting at the prefix offset.
  - PV 2-way column-packed (col groups at partitions 0 and 64, 33-wide
    V|ones weights); denominator via the ones column; groups merged in the
    tail with one DVE add.
  - Normalization without transposes: per-block column sums -> reshape DMA
    [8,64] -> reciprocal -> [1,512] -> K=1 replicate matmul -> tensor_mul.
    Output stays in O^T layout [32, 4096] per core.
"""

import math

import numpy as np

import concourse.bass as bass
import concourse.mybir as mybir
from concourse import bacc
from concourse.tile import TileContext
from concourse.bass_utils import run_bass_kernel_spmd

# Problem constants (hardcoded per harness contract).
B, CQ, CK, CH, NH, H, W = 1, 256, 256, 256, 8, 64, 64
DH = CH // NH            # 32
N = H * W                # 4096
QB = 512                 # queries per block
NQB = N // QB            # 8
KT = 128                 # keys per k-tile
NKT = N // KT            # 32
GS = 3                   # k-tiles per score group (3 PSUM banks, 3-way row pack)
SCALE = 1.0 / math.sqrt(DH)
NWARM = 10               # PE warmup matmuls (HAM clock-gate warmup)

F32 = mybir.dt.float32
F32R = mybir.dt.float32r
F16 = mybir.dt.float16

_CACHED_NC = None


def _build():
    nc = bacc.Bacc("TRN2", target_bir_lowering=False, debug=False, num_devices=1)

    qin_d = nc.dram_tensor("qin", [CQ, N], F32, kind="ExternalInput")
    kin_d = nc.dram_tensor("kin", [CK, N], F32, kind="ExternalInput")
    wq_d = nc.dram_tensor("wqt3", [CQ, 96], F32, kind="ExternalInput")
    wk_d = nc.dram_tensor("wkt3", [CK, 96], F32, kind="ExternalInput")
    wv_d = nc.dram_tensor("wvt", [CK, DH], F32, kind="ExternalInput")
    bq_d = nc.dram_tensor("bq3", [96, 1], F32, kind="ExternalInput")
    bk_d = nc.dram_tensor("bk3", [96, 1], F32, kind="ExternalInput")
    bv_d = nc.dram_tensor("bvr", [128, 4 * DH], F32, kind="ExternalInput")
    out_d = nc.dram_tensor("out", [DH, N], F32, kind="ExternalOutput")

    # Strict-causal within-tile mask: tm[kk, qq] = 1.0 iff kk < qq.
    tm_np = (np.arange(128)[:, None] < np.arange(128)[None, :]).astype(np.float16)
    tm_d = nc.inline_tensor(tm_np, name="tmask")
    ones_d = nc.inline_tensor(np.ones((1, DH), dtype=np.float32), name="onesd")

    kin_ap = kin_d.ap().rearrange("(c p) n -> p c n", p=128).bitcast(F32R)
    qin_ap = qin_d.ap().rearrange("(c p) n -> p c n", p=128).bitcast(F32R)
    wk_ap = wk_d.ap().rearrange("(c p) m -> p c m", p=128).bitcast(F32R)
    wq_ap = wq_d.ap().rearrange("(c p) m -> p c m", p=128).bitcast(F32R)
    wv_ap = wv_d.ap().rearrange("(c p) m -> p c m", p=128).bitcast(F32R)

    with TileContext(nc) as tc:
        with (
            tc.tile_pool(name="constp", bufs=1) as constp,
            tc.tile_pool(name="bigp", bufs=1) as bigp,
            tc.tile_pool(name="workp", bufs=4) as workp,
            tc.tile_pool(name="spool", bufs=2, space="PSUM") as spool,
            tc.tile_pool(name="mpool", bufs=2, space="PSUM") as mpool,
        ):
            # ---- big SBUF tiles ----
            kin_sb = bigp.tile([128, 2, N], F32R, name="kin_sb")
            qin_sb = bigp.tile([128, 2, N], F32R, name="qin_sb")
            kT3 = bigp.tile([96, N], F16, name="kT3")    # 3x replicated k^T
            qT3 = bigp.tile([96, N], F16, name="qT3")    # 3x replicated q^T
            # v_all[kk, t, 0:DH] = v[128t+kk, :]; col DH is the ones column
            v_all = bigp.tile([128, NKT, 48], F16, name="v_all")
            warm = bigp.tile([32, 640], F16, name="warm")

            # ---- DMAs: weights + first slices first; kin on sync, qin on
            # gpsimd (cheap issue), nothing on the scalar queue ----
            wk_sb = constp.tile([128, 2, 96], F32R, name="wk_sb")
            nc.sync.dma_start(wk_sb[:], wk_ap)
            wq_sb = constp.tile([128, 2, 96], F32R, name="wq_sb")
            nc.sync.dma_start(wq_sb[:], wq_ap)
            slA = slice(0, QB)
            nc.sync.dma_start(kin_sb[:, :, slA], kin_ap[:, :, slA])
            nc.sync.dma_start(qin_sb[:, :, slA], qin_ap[:, :, slA])
            wv_sb = constp.tile([128, 2, DH], F32R, name="wv_sb")
            nc.gpsimd.dma_start(wv_sb[:], wv_ap)
            bq_sb = constp.tile([96, 1], F32, name="bq_sb")
            nc.gpsimd.dma_start(bq_sb[:], bq_d.ap())
            bk_sb = constp.tile([96, 1], F32, name="bk_sb")
            nc.gpsimd.dma_start(bk_sb[:], bk_d.ap())
            bv_sb = constp.tile([128, 4, DH], F32, name="bv_sb")
            nc.gpsimd.dma_start(bv_sb[:], bv_d.ap().rearrange("p (t d) -> p t d", t=4))
            tm_sb = constp.tile([128, 128], F16, name="tm_sb")
            nc.gpsimd.dma_start(tm_sb[:], tm_d.ap())
            ones_sb = constp.tile([1, DH], F32R, name="ones_sb")
            nc.gpsimd.dma_start(ones_sb[:], ones_d.ap().bitcast(F32R))
            nc.vector.memset(warm[:], 0.0)
            nc.vector.memset(v_all[:, :, DH : DH + 1], 1.0)
            for s in range(1, NQB):
                sl = slice(QB * s, QB * (s + 1))
                nc.gpsimd.dma_start(kin_sb[:, :, sl], kin_ap[:, :, sl])
                nc.gpsimd.dma_start(qin_sb[:, :, sl], qin_ap[:, :, sl])

            # ---- PE warmup: keep the array busy through the DMA window so
            # the HAM un-throttles before real matmuls arrive ----
            wm_ps = mpool.tile([128, 512], F32, name="wm_ps", tag="p")
            for i in range(NWARM):
                nc.tensor.matmul(
                    wm_ps[:], warm[0:32, 0:128], warm[0:32, 128:640],
                    start=(i == 0), stop=(i == NWARM - 1),
                )

            stage_q = []  # deferred tail stages, advanced one per score group

            def emit_kq(s):
                ksl = slice(QB * s, QB * (s + 1))
                pjk = mpool.tile([96, 512], F32, name="pjk", tag="p")
                for ch in range(2):
                    nc.tensor.matmul(
                        pjk[:], wk_sb[:, ch, :], kin_sb[:, ch, ksl],
                        start=(ch == 0), stop=(ch == 1),
                    )
                nc.vector.tensor_scalar_add(kT3[:, ksl], pjk[:], bk_sb[:])
                pjq = mpool.tile([96, 512], F32, name="pjq", tag="p")
                for ch in range(2):
                    nc.tensor.matmul(
                        pjq[:], wq_sb[:, ch, :], qin_sb[:, ch, ksl],
                        start=(ch == 0), stop=(ch == 1),
                    )
                nc.vector.tensor_scalar_add(qT3[:, ksl], pjq[:], bq_sb[:])

            def emit_v4(s):
                # 4 v-tiles of one slice into one PSUM tile: no per-tile DVE
                # round-trips on the single-bank "p" ring, one batched add
                pv4 = mpool.tile([128, 4, DH], F32, name="pv4", tag="p")
                for ti in range(4):
                    t = 4 * s + ti
                    nsl = slice(128 * t, 128 * (t + 1))
                    for ch in range(2):
                        nc.tensor.matmul(
                            pv4[:, ti, :], kin_sb[:, ch, nsl], wv_sb[:, ch, :],
                            start=(ch == 0), stop=(ch == 1),
                        )
                nc.vector.tensor_add(
                    v_all[:, 4 * s : 4 * s + 4, 0:DH], pv4[:], bv_sb[:]
                )

            def tail_b(st):
                cs8r = workp.tile([8, 64], F32, name="cs8r")
                nc.vector.reciprocal(cs8r[:], st["cs8"][:])
                csr = workp.tile([1, 512], F32R, name="csr")
                nc.sync.dma_start(csr[:], cs8r[:].bitcast(F32R))
                st.update(csr=csr)

            def tail_c(st):
                qb = st["qb"]
                rep_ps = mpool.tile([DH, 512], F32, name="rep_ps", tag="p")
                nc.tensor.matmul(
                    rep_ps[:], ones_sb[:], st["csr"][:], start=True, stop=True
                )
                out_sb = workp.tile([DH, 512], F32, name="out_sb")
                nc.vector.tensor_mul(out_sb[:], st["o_sb"][:], rep_ps[:])
                nc.sync.dma_start(
                    out_d.ap()[:, QB * qb : QB * (qb + 1)], out_sb[:]
                )

            def emit_qb(qb):
                nkt = 4 * (qb + 1)
                o_ps = mpool.tile([DH + 1, 512], F32, name="o_ps", tag="o", bufs=1)
                pends = []

                def flush_pv(pend):
                    tiles, p_sb = pend
                    for (u, j) in tiles:
                        o = max(0, 128 * j - QB * qb)
                        nc.tensor.matmul(
                            o_ps[:, o:512],
                            v_all[:, j, 0 : DH + 1],
                            p_sb[:, 512 * u + o : 512 * (u + 1)],
                            start=(j == 0),
                            stop=(j == nkt - 1),
                            skip_group_check=True,
                        )

                # alternating 3-bank / 2-bank score groups: ping-pong
                # double-buffering with 5 PSUM banks
                sizes = []
                rem = nkt
                while rem > 0:
                    cap = 3 if len(sizes) % 2 == 0 else 2
                    sz = min(cap, rem)
                    sizes.append(sz)
                    rem -= sz
                jbase = 0
                for g, sz in enumerate(sizes):
                    tiles = [(u, jbase + u) for u in range(sz)]
                    jbase += sz
                    if g % 2 == 0:
                        s_ps = spool.tile(
                            [128, 3 * 512], F32, name="s_psA", tag="sA", bufs=1
                        )
                    else:
                        s_ps = spool.tile(
                            [128, 2 * 512], F32, name="s_psB", tag="sB", bufs=1
                        )
                    for (u, j) in tiles:
                        o = max(0, 128 * j - QB * qb)
                        nc.tensor.matmul(
                            s_ps[:, 512 * u + o : 512 * (u + 1)],
                            kT3[32 * u : 32 * u + 32, 128 * j : 128 * (j + 1)],
                            qT3[32 * u : 32 * u + 32, QB * qb + o : QB * (qb + 1)],
                            start=True, stop=True,
                        )
                    p_sb = workp.tile([128, 3 * 512], F16, name="p_sb", bufs=8)
                    nc.scalar.activation(
                        p_sb[:, 0 : 512 * sz],
                        s_ps[:, 0 : 512 * sz],
                        mybir.ActivationFunctionType.Exp,
                        scale=SCALE,
                    )
                    for (u, j) in tiles:
                        o = 128 * j - QB * qb
                        if o >= 0:  # strict-causal mask on the diagonal window
                            nc.gpsimd.tensor_mul(
                                p_sb[:, 512 * u + o : 512 * u + o + 128],
                                p_sb[:, 512 * u + o : 512 * u + o + 128],
                                tm_sb[:],
                            )
                    pends.append((tiles, p_sb))
                    if len(pends) > 2:
                        flush_pv(pends.pop(0))
                    if stage_q:
                        stage_q.pop(0)()
                while pends:
                    flush_pv(pends.pop(0))

                # tail_a inline
                o_sb = workp.tile([DH, 512], F32, name="o_sb")
                cs_sb = workp.tile([1, 512], F32, name="cs_sb")
                nc.vector.tensor_copy(o_sb[:], o_ps[0:DH, :])
                # +1e-30 keeps q=0 (fully masked row) at 0 instead of NaN
                nc.vector.tensor_scalar_add(cs_sb[:], o_ps[DH : DH + 1, :], 1e-30)
                cs8 = workp.tile([8, 64], F32, name="cs8")
                nc.sync.dma_start(cs8[:], cs_sb[:])
                st = {"qb": qb, "o_sb": o_sb, "cs8": cs8}
                stage_q.append(lambda st=st: tail_b(st))
                stage_q.append(lambda: None)
                stage_q.append(lambda st=st: tail_c(st))

            emit_kq(0)
            emit_kq(1)
            emit_v4(0)
            for qb in range(NQB):
                emit_qb(qb)
                if qb + 2 < NQB:
                    emit_kq(qb + 2)
                if qb + 1 < NQB:
                    emit_v4(qb + 1)
            while stage_q:
                stage_q.pop(0)()

    nc.finalize()
    return nc


def _get_nc():
    global _CACHED_NC
    if _CACHED_NC is None:
        _CACHED_NC = _build()
    return _CACHED_NC


def _prep_in_maps(inputs):
    f = lambda a: np.ascontiguousarray(np.asarray(a, dtype=np.float32))
    query = f(inputs["query"]).reshape(CQ, N)
    key_feat = f(inputs["key_feat"]).reshape(CK, N)

    def wnorm(v, g):
        v = f(v)
        g = f(g)
        return g[:, None] * v / np.linalg.norm(v, axis=1, keepdims=True)

    wq = wnorm(inputs["vq"], inputs["gq"])
    wk = wnorm(inputs["vk"], inputs["gk"])
    wv = wnorm(inputs["vv"], inputs["gv"])
    bq, bk, bv = f(inputs["bq"]), f(inputs["bk"]), f(inputs["bv"])

    in_maps = []
    for c in range(NH):
        rows = slice(DH * c, DH * (c + 1))
        in_maps.append(
            {
                "qin": query,
                "kin": key_feat,
                "wqt3": np.ascontiguousarray(np.tile(wq[rows].T, (1, 3))),
                "wkt3": np.ascontiguousarray(np.tile(wk[rows].T, (1, 3))),
                "wvt": np.ascontiguousarray(wv[rows].T),
                "bq3": np.ascontiguousarray(np.tile(bq[rows], 3)[:, None]),
                "bk3": np.ascontiguousarray(np.tile(bk[rows], 3)[:, None]),
                "bvr": np.ascontiguousarray(np.tile(bv[rows][None, :], (128, 4))),
            }
        )
    return in_maps


def _run(inputs, trace=False, **kwargs):
    nc = _get_nc()
    in_maps = _prep_in_maps(inputs)
    res = None
    for attempt in range(3):
        try:
            res = run_bass_kernel_spmd(
                nc, in_maps, core_ids=list(range(NH)), trace=trace, **kwargs
            )
            break
        except Exception:
            if attempt == 2:
                raise

    out = np.empty((B, CH, H, W), dtype=np.float32)
    for c in range(NH):
        oc = res.results[c]["out"]  # [DH, N] (O^T layout)
        out[0, DH * c : DH * (c + 1)] = oc.reshape(DH, H, W)
    return out, res


def kernel(**inputs) -> np.ndarray:
    out, _ = _run(inputs, trace=False)
    return out
